# revision 22
# baseline (speedup 1.0000x reference)
import sys

sys.path.insert(0, "/opt/trn_rl_repo")

import numpy as np  # noqa: E402
import ml_dtypes  # noqa: E402

import concourse.bass as bass  # noqa: E402
from concourse.bass_isa import ReduceOp  # noqa: E402
import concourse.mybir as mybir  # noqa: E402
import concourse.tile as tile  # noqa: E402
from contextlib import ExitStack  # noqa: E402
from concourse import bacc  # noqa: E402
from concourse.bass_utils import run_bass_kernel_spmd  # noqa: E402
from concourse.masks import make_identity  # noqa: E402

F32 = mybir.dt.float32
BF16 = mybir.dt.bfloat16
AF = mybir.ActivationFunctionType
ALU = mybir.AluOpType
AX = mybir.AxisListType
NPBF = ml_dtypes.bfloat16

S = 4  # samples per core
C, H, W = 256, 28, 28
N = H * W  # 784
NK = 196
HEADS, DK = 8, 32
CM = 1024
SCALE = DK ** -0.5
EPS = 1e-5
INV_NTOT = 1.0 / (C * N)
ISL = [(0, 512), (512, 272)]  # bank-aligned free splits of 784
NCORES = 8

# ---- boot bf16 pack (everything needed before the FFN) ----
B_WQ = 0                      # 2 * 256
B_WK = B_WQ + 512
B_WV = B_WK + 512
B_WO = B_WV + 512
B_BH = B_WO + 512             # 4 * 128
B_ONE = B_BH + 512            # 128 (ones)
B_BOR = B_ONE + 128           # 256 (row0 only)
B_BVR = B_BOR + 256           # 256 (row0 only)
BCOLS = B_BVR + 256

# ---- FFN bf16 pack column offsets ----
O_DGDW2 = 0                   # 72 * 128
O_C1 = O_DGDW2 + 72 * 128     # 2 * 1024
O_C2 = O_C1 + 2048            # 8 * 256
WCOLS = O_C2 + 2048

# ---- f32 const-pack column offsets ----
P_ONEM = 0                  # 128 (ones, f32)
P_A1 = P_ONEM + 128         # 8
P_B1 = P_A1 + 8
P_A2 = P_B1 + 8
P_B2 = P_A2 + 8
P_A3 = P_B2 + 8             # 2
P_B3 = P_A3 + 2
P_LPUB = P_B3 + 2           # 2
P_DWB = P_LPUB + 2
P_BQ = P_DWB + 2
P_BK = P_BQ + 2
P_ZERO = P_BK + 2           # 1
P_EPSC = P_ZERO + 1         # 1
P_CS = P_EPSC + 1           # 1 (1.5 * seed const)
P_C15 = P_CS + 1            # 1 (1.5)
P_BOB = P_C15 + 1           # 256 bo broadcast
P_BVB = P_BOB + 256         # 256 bv broadcast
P_W1S = P_BVB + 256         # 8: row sums of bf16(c1_w)
P_LPUW = P_W1S + 8          # 18: lpu_w taps per channel-block
P_KVW = P_LPUW + 18         # 8: dw_w taps
FCOLS = P_KVW + 8

_CACHE = {}


def _prep(inputs):
    """Host-side precompute: all weight-derived constants in SBUF-ready
    layouts so the device program only DMAs them in."""
    f32 = np.float32
    bpk = np.zeros((128, BCOLS), dtype=NPBF)
    wpk = np.zeros((128, WCOLS), dtype=NPBF)
    fpk = np.zeros((128, FCOLS), dtype=f32)

    def put_diags(dst, off, w2d, G, T):
        for g in range(G):
            for t in range(T):
                d = np.zeros((128, 128), f32)
                np.fill_diagonal(d, w2d[g * 128:(g + 1) * 128, t])
                k = off + (g * T + t) * 128
                dst[:, k:k + 128] = d.astype(NPBF)

    put_diags(wpk, O_DGDW2, np.asarray(inputs["dw2_w"], f32).reshape(CM, 9), 8, 9)

    def put_wT(off, w, km, scale=1.0, dst=None):
        # w [M, K] -> km tiles [128, M] ; tile kc = w[:, kc*128:+128].T
        if dst is None:
            dst = wpk
        wT = (np.asarray(w, f32).T * scale).astype(NPBF)  # [K, M]
        M = wT.shape[1]
        for kc in range(km):
            dst[:, off + kc * M:off + (kc + 1) * M] = wT[kc * 128:(kc + 1) * 128]

    put_wT(B_WQ, inputs["wq"], 2, SCALE, dst=bpk)
    put_wT(B_WK, inputs["wk"], 2, dst=bpk)
    put_wT(B_WV, inputs["wv"], 2, dst=bpk)
    put_wT(B_WO, inputs["wo"], 2, dst=bpk)
    put_wT(O_C1, np.asarray(inputs["c1_w"], f32).reshape(CM, C), 2)
    put_wT(O_C2, np.asarray(inputs["c2_w"], f32).reshape(C, CM), 8)

    for q in range(4):
        bpk[:, B_BH + q * 128 + 32 * q:B_BH + q * 128 + 32 * q + 32] = NPBF(1.0)
    bpk[:, B_ONE:B_ONE + 128] = NPBF(1.0)
    bpk[0, B_BOR:B_BOR + 256] = np.asarray(inputs["bo"], f32).astype(NPBF)
    bpk[0, B_BVR:B_BVR + 256] = np.asarray(inputs["bv"], f32).astype(NPBF)

    # exp(pos_b)^T per head, packed [128, 1568] each
    pos = np.asarray(inputs["pos_b"], f32)[0]          # [8, 784, 196]
    e = np.exp(pos.transpose(0, 2, 1))                 # [8, 196, 784]
    et = np.zeros((128, HEADS * 2 * N), dtype=NPBF)
    for h in range(HEADS):
        et[:, h * 2 * N:h * 2 * N + N] = e[h, :128].astype(NPBF)
        et[:68, h * 2 * N + N:(h + 1) * 2 * N] = e[h, 128:].astype(NPBF)

    fpk[:, P_ONEM:P_ONEM + 128] = 1.0

    def bn_fold(offA, offB, g, b, m, v, cb, G):
        g, b, m, v, cb = (np.asarray(t, f32) for t in (g, b, m, v, cb))
        A = g / np.sqrt(v + EPS)
        B = b - m * A + A * cb
        fpk[:, offA:offA + G] = A.reshape(G, 128).T
        fpk[:, offB:offB + G] = B.reshape(G, 128).T

    bn_fold(P_A1, P_B1, inputs["bn1_g"], inputs["bn1_b"], inputs["bn1_m"],
            inputs["bn1_v"], inputs["c1_b"], 8)
    bn_fold(P_A2, P_B2, inputs["bn2_g"], inputs["bn2_b"], inputs["bn2_m"],
            inputs["bn2_v"], inputs["dw2_b"], 8)
    bn_fold(P_A3, P_B3, inputs["bn3_g"], inputs["bn3_b"], inputs["bn3_m"],
            inputs["bn3_v"], inputs["c2_b"], 2)

    fpk[:, P_LPUB:P_LPUB + 2] = np.asarray(inputs["lpu_b"], f32).reshape(2, 128).T
    fpk[:, P_DWB:P_DWB + 2] = np.asarray(inputs["dw_b"], f32).reshape(2, 128).T
    fpk[:, P_BQ:P_BQ + 2] = (np.asarray(inputs["bq"], f32) * SCALE).reshape(2, 128).T
    fpk[:, P_BK:P_BK + 2] = np.asarray(inputs["bk"], f32).reshape(2, 128).T
    fpk[:, P_ZERO] = 0.0
    fpk[:, P_EPSC] = EPS
    fpk[:, P_CS] = 1.5 * 0.78
    fpk[:, P_C15] = 1.5
    fpk[:, P_BOB:P_BOB + 256] = np.asarray(inputs["bo"], f32)[None, :]
    fpk[:, P_BVB:P_BVB + 256] = np.asarray(inputs["bv"], f32)[None, :]
    c1bf = np.asarray(inputs["c1_w"], f32).reshape(CM, C).astype(NPBF)
    fpk[:, P_W1S:P_W1S + 8] = c1bf.astype(f32).sum(axis=1).reshape(8, 128).T
    lw = np.asarray(inputs["lpu_w"], f32).reshape(2, 128, 9)
    fpk[:, P_LPUW:P_LPUW + 18] = np.concatenate([lw[0], lw[1]], axis=1)
    kw = np.asarray(inputs["dw_w"], f32).reshape(2, 128, 4)
    fpk[:, P_KVW:P_KVW + 8] = np.concatenate([kw[0], kw[1]], axis=1)

    return bpk, wpk, fpk, et


def _build():
    if "nc" in _CACHE:
        return _CACHE["nc"]
    nc = bacc.Bacc()

    x_d = nc.dram_tensor("x", [S, C, H, W], F32, kind="ExternalInput")
    y_d = nc.dram_tensor("y", [S, C, H, W], F32, kind="ExternalOutput")
    scr_d = nc.dram_tensor("scr", [S, N * C], F32)
    bpk_d = nc.dram_tensor("bpk", [128, BCOLS], BF16, kind="ExternalInput")
    wpk_d = nc.dram_tensor("wpk", [128, WCOLS], BF16, kind="ExternalInput")
    fpk_d = nc.dram_tensor("fpk", [128, FCOLS], F32, kind="ExternalInput")
    et_d = nc.dram_tensor("et", [128, HEADS * 2 * N], BF16, kind="ExternalInput")

    xv = x_d.rearrange("s c h w -> s c (h w)")
    yv = y_d.rearrange("s c h w -> s c (h w)")

    with tile.TileContext(nc) as tc, ExitStack() as stk:
        cst = stk.enter_context(tc.tile_pool(name="cst", bufs=1))
        wk2 = stk.enter_context(tc.tile_pool(name="wk2", bufs=2))
        wk1 = stk.enter_context(tc.tile_pool(name="wk1", bufs=1))
        psA = stk.enter_context(tc.tile_pool(name="psA", bufs=3, space="PSUM"))
        psS = stk.enter_context(tc.tile_pool(name="psS", bufs=2, space="PSUM"))

        def pat(shape=(128, N)):
            return psA.tile(list(shape), F32, tag="attn", name="pat")

        def psm(shape=(128, 392)):
            return psS.tile(list(shape), F32, tag="small", name="psm")

        # ---------- resident constants (pure DMA) ----------
        fpk = cst.tile([128, FCOLS], F32, tag="fpk")
        nc.scalar.dma_start(out=fpk, in_=fpk_d[:, :])
        bpkt = cst.tile([128, BCOLS], BF16, tag="bpkt")
        nc.scalar.dma_start(out=bpkt, in_=bpk_d[:, :])
        et = cst.tile([128, HEADS * 2 * N], BF16, tag="et")
        wpk = cst.tile([128, WCOLS], BF16, tag="wpk")
        nc.gpsimd.dma_start(out=wpk, in_=wpk_d[:, :])

        def wcol(off, w):
            return wpk[:, off:off + w]

        # lpu/kv diag matrices built on-chip (ident * per-partition tap)
        ident = cst.tile([128, 128], BF16, tag="ident")
        make_identity(nc, ident)
        dgb = cst.tile([128, 26 * 128], BF16, tag="dgb")
        dg_lpu, dg_kv = [], []
        for g in range(2):
            row = []
            for t in range(9):
                k = (g * 9 + t) * 128
                nc.vector.tensor_scalar(
                    out=dgb[:, k:k + 128], in0=ident,
                    scalar1=fpk[:, P_LPUW + g * 9 + t:P_LPUW + g * 9 + t + 1],
                    scalar2=None, op0=ALU.mult)
                row.append(dgb[:, k:k + 128])
            dg_lpu.append(row)
        for g in range(2):
            row = []
            for t in range(4):
                k = (18 + g * 4 + t) * 128
                nc.vector.tensor_scalar(
                    out=dgb[:, k:k + 128], in0=ident,
                    scalar1=fpk[:, P_KVW + g * 4 + t:P_KVW + g * 4 + t + 1],
                    scalar2=None, op0=ALU.mult)
                row.append(dgb[:, k:k + 128])
            dg_kv.append(row)
        dg_dw2 = [[wcol(O_DGDW2 + (g * 9 + t) * 128, 128) for t in range(9)]
                  for g in range(8)]
        wqT = [bpkt[:, B_WQ + kc * 256:B_WQ + kc * 256 + 256] for kc in range(2)]
        wkT = [bpkt[:, B_WK + kc * 256:B_WK + kc * 256 + 256] for kc in range(2)]
        wvT = [bpkt[:, B_WV + kc * 256:B_WV + kc * 256 + 256] for kc in range(2)]
        woT = [bpkt[:, B_WO + kc * 256:B_WO + kc * 256 + 256] for kc in range(2)]
        c1T = [wcol(O_C1 + kc * 1024, 1024) for kc in range(2)]
        c2T = [wcol(O_C2 + kc * 256, 256) for kc in range(8)]
        bh = [bpkt[:, B_BH + q * 128:B_BH + q * 128 + 128] for q in range(4)]
        ones1 = bpkt[0:1, B_ONE:B_ONE + 128]
        bo_r = bpkt[0:1, B_BOR:B_BOR + 256]
        bv_r = bpkt[0:1, B_BVR:B_BVR + 256]
        Ec = [et[:, h * 2 * N:(h + 1) * 2 * N] for h in range(HEADS)]

        onesM = fpk[:, P_ONEM:P_ONEM + 128]
        A1 = fpk[:, P_A1:P_A1 + 8]; B1 = fpk[:, P_B1:P_B1 + 8]
        A2 = fpk[:, P_A2:P_A2 + 8]; B2 = fpk[:, P_B2:P_B2 + 8]
        A3 = fpk[:, P_A3:P_A3 + 2]; B3 = fpk[:, P_B3:P_B3 + 2]
        lpub = [fpk[:, P_LPUB + g:P_LPUB + g + 1] for g in range(2)]
        zcol = fpk[:, P_ZERO:P_ZERO + 1]
        epsc = fpk[:, P_EPSC:P_EPSC + 1]
        cscol = fpk[:, P_CS:P_CS + 1]
        c15col = fpk[:, P_C15:P_C15 + 1]
        bob = fpk[:, P_BOB:P_BOB + 256]
        bvb = fpk[:, P_BVB:P_BVB + 256]
        w1s = fpk[:, P_W1S:P_W1S + 8]
        dwb = [fpk[:, P_DWB + g:P_DWB + g + 1] for g in range(2)]
        bqc = [fpk[:, P_BQ + g:P_BQ + g + 1] for g in range(2)]
        bkc = [fpk[:, P_BK + g:P_BK + g + 1] for g in range(2)]

        # LN over (C,H,W): mean via reduce, rstd via table-free NR rsqrt
        # (Identity/Square activations only -> no ACT table switches).
        def ln_stats_pre(chunks, tagp):
            st4 = wk2.tile([128, 4], F32, tag=f"st4{tagp}", name="st4")
            for ch in range(2):
                nc.vector.tensor_reduce(
                    out=st4[:, 2 * ch:2 * ch + 1], in_=chunks[ch], axis=AX.X,
                    op=ALU.add)
                scr = wk2.tile([128, N], BF16, tag="lnsc", name="scr")
                nc.scalar.activation(
                    out=scr, in_=chunks[ch], func=AF.Square,
                    accum_out=st4[:, 2 * ch + 1:2 * ch + 2])
            return st4

        def ln_stats_post(st4, tagp):
            stc = wk2.tile([128, 4], F32, tag=f"stc{tagp}", name="stc")
            nc.gpsimd.partition_all_reduce(stc, st4, 128, ReduceOp.add)
            tot = wk2.tile([128, 2], F32, tag=f"tot{tagp}", name="tot")
            nc.vector.tensor_add(out=tot, in0=stc[:, 0:2], in1=stc[:, 2:4])
            # scalar-engine chain: only Identity/Square (in every act table,
            # so no ACT_TABLE_LOAD thrash). scale/bias APs give tensor*tensor.
            mv2 = wk2.tile([128, 2], F32, tag=f"mv2{tagp}", name="mv2")
            nc.scalar.activation(out=mv2, in_=tot, func=AF.Identity,
                                 scale=INV_NTOT, bias=epsc)
            mean = mv2[:, 0:1]
            m2 = wk2.tile([128, 1], F32, tag=f"m2{tagp}", name="m2")
            nc.scalar.activation(out=m2, in_=mean, func=AF.Square)
            var = wk2.tile([128, 1], F32, tag=f"var{tagp}", name="var")
            nc.scalar.activation(out=var, in_=m2, func=AF.Identity,
                                 scale=-1.0, bias=mv2[:, 1:2])
            # table-free rsqrt: fixed-seed Newton iterations (var ~ O(1))
            ya = wk2.tile([128, 1], F32, tag=f"ya{tagp}", name="ya")
            yb = wk2.tile([128, 1], F32, tag=f"yb{tagp}", name="yb")
            y2 = wk2.tile([128, 1], F32, tag=f"y2{tagp}", name="y2")
            a = wk2.tile([128, 1], F32, tag=f"a{tagp}", name="a")
            nc.scalar.activation(out=ya, in_=var, func=AF.Identity,
                                 scale=-0.5 * 0.78 ** 3, bias=cscol)
            cur, nxt = ya, yb
            for _ in range(2):
                nc.scalar.activation(out=y2, in_=cur, func=AF.Square)
                nc.scalar.activation(out=y2, in_=y2, func=AF.Identity,
                                     scale=var)
                nc.scalar.activation(out=a, in_=y2, func=AF.Identity,
                                     scale=-0.5, bias=c15col)
                nc.scalar.activation(out=nxt, in_=a, func=AF.Identity,
                                     scale=cur)
                cur, nxt = nxt, cur
            return mean, cur

        # ---------- per-sample stages ----------
        def front_a0(s):
            """x loads + padded bf16 copies (cheap vector work only)"""
            st = {}
            xs = []
            for ch in range(2):
                t = wk2.tile([128, N], F32, tag="xs", name="t")
                nc.sync.dma_start(out=t, in_=xv[s, ch * 128:(ch + 1) * 128, :])
                xs.append(t)
            if s == 0:
                half = HEADS * N
                nc.sync.dma_start(out=et[:, :half], in_=et_d[:, :half])
                nc.scalar.dma_start(out=et[:, half:], in_=et_d[:, half:])
            xb = []
            for ch in range(2):
                p = wk2.tile([128, 30, 30], BF16, tag=f"xb{ch}", bufs=1, name="p")
                if s == 0:
                    nc.vector.memset(p, 0.0)
                nc.vector.tensor_copy(
                    out=p[:, 1:29, 1:29],
                    in_=xs[ch].rearrange("p (h w) -> p h w", w=W))
                xb.append(p)
            st["xs"], st["xb"] = xs, xb
            return st

        def front_a(s, st):
            """LPU + kv conv + k/v projections"""
            xs, xb = st["xs"], st["xb"]
            x1, x1b = [], []
            for ch in range(2):
                t = wk2.tile([128, N], F32, tag=f"x1{ch}", name="t")
                for hf in range(2):
                    pl = pat((128, 392))
                    for t9 in range(9):
                        dy, dx = t9 // 3, t9 % 3
                        nc.tensor.matmul(
                            pl, dg_lpu[ch][t9],
                            xb[ch][:, dy + 14 * hf:dy + 14 * hf + 14, dx:dx + 28],
                            start=(t9 == 0), stop=(t9 == 8))
                    nc.vector.scalar_tensor_tensor(
                        out=t[:, hf * 392:(hf + 1) * 392], in0=pl,
                        scalar=lpub[ch], in1=xs[ch][:, hf * 392:(hf + 1) * 392],
                        op0=ALU.add, op1=ALU.add)
                x1.append(t)
                tb = wk2.tile([128, N], BF16, tag=f"x1b{ch}", name="tb")
                nc.vector.tensor_copy(out=tb, in_=t)
                x1b.append(tb)
            kvb = []
            for ch in range(2):
                x5 = x1b[ch].rearrange(
                    "p (h a w b) -> p h a w b", h=14, a=2, w=14, b=2)
                pk = pat((128, NK))
                for t4 in range(4):
                    dy, dx = t4 // 2, t4 % 2
                    nc.tensor.matmul(
                        pk, dg_kv[ch][t4], x5[:, :, dy, :, dx],
                        start=(t4 == 0), stop=(t4 == 3))
                t = wk2.tile([128, NK], BF16, tag=f"kvb{ch}", name="t")
                nc.scalar.activation(out=t, in_=pk, func=AF.Identity, bias=dwb[ch])
                kvb.append(t)
            st["x1"], st["kvb"] = x1, kvb
            kb = []
            for mc in range(2):
                pk2 = psm((128, NK))
                for kc in range(2):
                    nc.tensor.matmul(
                        pk2, wkT[kc][:, mc * 128:(mc + 1) * 128], kvb[kc],
                        start=(kc == 0), stop=(kc == 1))
                t = wk2.tile([128, NK], BF16, tag=f"kb{mc}", name="t")
                nc.scalar.activation(out=t, in_=pk2, func=AF.Identity,
                                     bias=bkc[mc])
                kb.append(t)
            vb = []
            for pi, (j0, jw) in enumerate([(0, 128), (128, 68)]):
                pv = psm((128, C))
                for kc in range(2):
                    nc.tensor.matmul(
                        pv[0:jw, :], kvb[kc][:, j0:j0 + jw], wvT[kc],
                        start=(kc == 0), stop=(kc == 1))
                t = wk2.tile([128, C], BF16, tag=f"vb{pi}", name="t")
                nc.vector.tensor_add(out=t[0:jw, :], in0=pv[0:jw, :],
                                     in1=bvb[0:jw, :])
                vb.append(t)
            st["kb"], st["vb"] = kb, vb
            return st

        def front_a15(s, st):
            st["st4l1"] = ln_stats_pre(st["x1"], "l1")

        def front_a2(s, st):
            st["ln1"] = ln_stats_post(st["st4l1"], "l1")

        def front_b(s, st):
            """LN1 normalize + q projection"""
            mean1, rstd1 = st["ln1"]
            x1 = st["x1"]
            ln1b = []
            for ch in range(2):
                t = wk2.tile([128, N], BF16, tag=f"ln1b{ch}", name="t")
                nc.vector.tensor_scalar(
                    out=t, in0=x1[ch], scalar1=mean1, scalar2=rstd1,
                    op0=ALU.subtract, op1=ALU.mult)
                ln1b.append(t)
            qb = []
            for mc in range(2):
                pq = pat()
                for i0, iw in ISL:
                    for kc in range(2):
                        nc.tensor.matmul(
                            pq[:, i0:i0 + iw],
                            wqT[kc][:, mc * 128:(mc + 1) * 128],
                            ln1b[kc][:, i0:i0 + iw],
                            start=(kc == 0), stop=(kc == 1))
                t = wk2.tile([128, N], BF16, tag=f"qb{mc}", name="t")
                nc.vector.tensor_scalar(
                    out=t, in0=pq, scalar1=bqc[mc], scalar2=None, op0=ALU.add)
                qb.append(t)
            st["qb"] = qb

        def attn(s, st):
            kb, vb, qb = st["kb"], st["vb"], st["qb"]
            # F1: QK^T + exp + E-mult per head
            paA, paB = [], []
            for h in range(HEADS):
                tc4, ro = h // 4, 32 * (h % 4)
                attA = pat()
                attB = pat()
                for i0, iw in ISL:
                    nc.tensor.matmul(
                        attA[:, i0:i0 + iw], kb[tc4][ro:ro + 32, 0:128],
                        qb[tc4][ro:ro + 32, i0:i0 + iw], start=True, stop=True,
                        tile_position=(ro, 0))
                    nc.tensor.matmul(
                        attB[0:68, i0:i0 + iw], kb[tc4][ro:ro + 32, 128:NK],
                        qb[tc4][ro:ro + 32, i0:i0 + iw], start=True, stop=True,
                        tile_position=(ro, 0))
                pA = wk1.tile([128, N], BF16, tag=f"paA{h}", name="pA")
                nc.scalar.activation(out=pA, in_=attA, func=AF.Exp)
                nc.vector.tensor_mul(out=pA, in0=pA, in1=Ec[h][:, 0:N])
                pB = wk1.tile([128, N], BF16, tag=f"paB{h}", name="pB")
                nc.scalar.activation(out=pB[0:68, :], in_=attB[0:68, :],
                                     func=AF.Exp)
                nc.vector.tensor_mul(
                    out=pB[0:68, :], in0=pB[0:68, :], in1=Ec[h][0:68, N:2 * N])
                paA.append(pA)
                paB.append(pB)

            # F2/F3 per tc4 group, interleaved to keep PE fed
            rS, tun, tnb = [None, None], [None, None], [None, None]
            for tc4 in range(2):
                S_ps = pat()
                for i0, iw in ISL:
                    for qq in range(4):
                        h = tc4 * 4 + qq
                        nc.tensor.matmul(
                            S_ps[:, i0:i0 + iw], bh[qq][0:128, :],
                            paA[h][:, i0:i0 + iw], start=(qq == 0), stop=False)
                        nc.tensor.matmul(
                            S_ps[:, i0:i0 + iw], bh[qq][0:68, :],
                            paB[h][0:68, i0:i0 + iw], start=False,
                            stop=(qq == 3))
                r = wk2.tile([128, N], F32, tag="rS", name="r")
                nc.vector.reciprocal_approx_fast(out=r, in_=S_ps)
                rS[tc4] = r
                tn = pat()
                for qq in range(4):
                    h = tc4 * 4 + qq
                    ro = 32 * qq
                    for i0, iw in ISL:
                        nc.tensor.matmul(
                            tn[ro:ro + 32, i0:i0 + iw],
                            vb[0][0:128, 32 * h:32 * h + 32],
                            paA[h][:, i0:i0 + iw], start=True, stop=False,
                            tile_position=(0, ro))
                        nc.tensor.matmul(
                            tn[ro:ro + 32, i0:i0 + iw],
                            vb[1][0:68, 32 * h:32 * h + 32],
                            paB[h][0:68, i0:i0 + iw], start=False, stop=True,
                            tile_position=(0, ro))
                t = wk2.tile([128, N], BF16, tag=f"tnb{tc4}", name="t")
                nc.vector.tensor_mul(out=t, in0=tn, in1=r)
                tnb[tc4] = t

            # F4: out-proj; DRAM round-trip does the raw reinterpret
            # ([98,256] o-chunk == rows 32j:32j+32 of [256,784])
            ore = [wk2.tile([128, N], F32, tag="ore0", name="ore0"),
                   wk2.tile([128, N], F32, tag="ore1", name="ore1")]
            for j in range(8):
                n0 = j * 98
                po = psm((128, C))
                for tc4 in range(2):
                    nc.tensor.matmul(
                        po[0:98, :], tnb[tc4][:, n0:n0 + 98], woT[tc4],
                        start=(tc4 == 0), stop=(tc4 == 1))
                osb = wk2.tile([128, C], F32, tag="osb", name="osb")
                nc.vector.tensor_add(out=osb[0:98, :], in0=po[0:98, :],
                                     in1=bob[0:98, :])
                nc.sync.dma_start(
                    out=scr_d[s, n0 * C:(n0 + 98) * C].rearrange(
                        "(n c) -> n c", c=C),
                    in_=osb[0:98, :])
                nc.scalar.dma_start(
                    out=ore[j // 4][32 * (j % 4):32 * (j % 4) + 32, :],
                    in_=scr_d[s, j * 25088:(j + 1) * 25088].rearrange(
                        "(a i) -> a i", i=N))
            st["ore"] = ore

        def back_stats1(s, st):
            x1, ore = st["x1"], st["ore"]
            x2 = []
            for ch in range(2):
                t = wk2.tile([128, N], F32, tag=f"x2{ch}", bufs=1, name="t")
                nc.vector.tensor_add(out=t, in0=ore[ch], in1=x1[ch])
                x2.append(t)
            st["x2"] = x2
            x2b = []
            for ch in range(2):
                t = wk2.tile([128, N], BF16, tag=f"x2b{ch}", name="t")
                nc.vector.tensor_copy(out=t, in_=x2[ch])
                x2b.append(t)
            st["x2b"] = x2b
            st["st4l2"] = ln_stats_pre(x2, "l2")

        def back_stats2(s, st):
            mean2, rstd2 = ln_stats_post(st["st4l2"], "l2")
            # fold LN2 into the c1->gelu affine: c1 ran on raw x2, so
            # gelu input = (A1*r)*h_pre + (B1 - A1*r*m*rowsum(c1_w))
            sc1 = wk2.tile([128, 8], F32, tag="sc1", name="sc1")
            nc.vector.tensor_scalar(
                out=sc1, in0=A1, scalar1=rstd2, scalar2=None, op0=ALU.mult)
            b1f = wk2.tile([128, 8], F32, tag="b1f", name="b1f")
            nc.vector.tensor_scalar(
                out=b1f, in0=w1s, scalar1=mean2, scalar2=None, op0=ALU.mult)
            nc.vector.tensor_mul(out=b1f, in0=b1f, in1=sc1)
            nc.vector.tensor_sub(out=b1f, in0=B1, in1=b1f)
            st["sc1"], st["b1f"] = sc1, b1f

        def back_ffn(s, st):
            x2, x2b = st["x2"], st["x2b"]
            sc1, b1f = st["sc1"], st["b1f"]
            h1p = []
            for mc in range(8):
                pc1 = pat()
                for i0, iw in ISL:
                    for kc in range(2):
                        nc.tensor.matmul(
                            pc1[:, i0:i0 + iw],
                            c1T[kc][:, mc * 128:(mc + 1) * 128],
                            x2b[kc][:, i0:i0 + iw],
                            start=(kc == 0), stop=(kc == 1))
                hp = wk1.tile([128, 30, 30], BF16, tag=f"h1p{mc}", name="hp")
                if s == 0:
                    nc.vector.memset(hp, 0.0)
                nc.scalar.activation(
                    out=hp[:, 1:29, 1:29],
                    in_=pc1.rearrange("p (h w) -> p h w", w=W),
                    func=AF.Gelu, scale=sc1[:, mc:mc + 1],
                    bias=b1f[:, mc:mc + 1])
                h1p.append(hp)
            pc2 = [pat(), pat()]
            for kc in range(8):
                ht = wk2.tile([128, N], BF16, tag="h2t", name="ht")
                for hf in range(2):
                    pd = psm()
                    for t9 in range(9):
                        dy, dx = t9 // 3, t9 % 3
                        nc.tensor.matmul(
                            pd, dg_dw2[kc][t9],
                            h1p[kc][:, dy + 14 * hf:dy + 14 * hf + 14,
                                    dx:dx + 28],
                            start=(t9 == 0), stop=(t9 == 8))
                    nc.scalar.activation(
                        out=ht[:, hf * 392:(hf + 1) * 392], in_=pd,
                        func=AF.Gelu, scale=A2[:, kc:kc + 1],
                        bias=B2[:, kc:kc + 1])
                for mc in range(2):
                    for i0, iw in ISL:
                        nc.tensor.matmul(
                            pc2[mc][:, i0:i0 + iw],
                            c2T[kc][:, mc * 128:(mc + 1) * 128],
                            ht[:, i0:i0 + iw],
                            start=(kc == 0), stop=(kc == 7))
            for mc in range(2):
                t3 = wk2.tile([128, N], F32, tag="t3", name="t3")
                nc.vector.tensor_scalar(
                    out=t3, in0=pc2[mc], scalar1=A3[:, mc:mc + 1],
                    scalar2=B3[:, mc:mc + 1], op0=ALU.mult, op1=ALU.add)
                nc.vector.tensor_add(out=t3, in0=t3, in1=x2[mc])
                nc.sync.dma_start(
                    out=yv[s, mc * 128:(mc + 1) * 128, :], in_=t3)

        # ---------- software-pipelined emission ----------
        states = {}
        states[0] = front_a0(0)
        front_a(0, states[0])
        front_a15(0, states[0])
        states[1] = front_a0(1)
        front_a(1, states[1])
        front_a2(0, states[0])
        front_b(0, states[0])
        for s in range(S):
            attn(s, states[s])
            if s + 2 < S:
                states[s + 2] = front_a0(s + 2)
            back_stats1(s, states[s])
            if s + 2 < S:
                front_a(s + 2, states[s + 2])
            if s + 1 < S:
                front_a15(s + 1, states[s + 1])
            back_stats2(s, states[s])
            back_ffn(s, states[s])
            del states[s]
            if s + 1 < S:
                front_a2(s + 1, states[s + 1])
                front_b(s + 1, states[s + 1])

    nc.finalize()
    _CACHE["nc"] = nc
    return nc


def _in_maps(inputs):
    bpk, wpk, fpk, et = _prep(inputs)
    x = np.ascontiguousarray(inputs["x"], dtype=np.float32)
    in_maps = []
    for c in range(NCORES):
        in_maps.append(dict(
            x=np.ascontiguousarray(x[c * S:(c + 1) * S]),
            bpk=bpk, wpk=wpk, fpk=fpk, et=et))
    return in_maps


def kernel(**inputs):
    nc = _build()
    res = run_bass_kernel_spmd(nc, _in_maps(inputs), core_ids=list(range(NCORES)))
    out = np.concatenate([res.results[c]["y"] for c in range(NCORES)], axis=0)
    return out
